# revision 5
# baseline (speedup 1.0000x reference)
"""FATM (wavelet spiking module) Trainium2 Bass kernel.

Data-parallel over B across 8 NeuronCores (B=8 -> 1 sample/core).

Per-core pipeline (validated in numpy against the reference):
  chunk-serial over 4 chunks of 128 channels, t-serial over T=4:
    LIF1 (A-layout, fp16 state in SBUF, scalar_tensor_tensor update)
    ST -> B-layout, BD4(Q) row/col matmuls -> NegIF1 state in PSUM
    NegIF spikes via two ACT Sign ops (s~ = sign(v-1)+sign(v+1) = 2s),
    membrane soft-reset via -0.5*I matmul feedback into PSUM
    bn0+LIF2 in C-layout with broadcast param tiles (threshold-offset trick
    folds the bn0 bias into per-(channel,t) thresholds + reset values)
    inverse Haar directly from C-layout, then channel-mix (bn1 folded into
    the mix weights; the bn1 bias enters as bias1[c]*(Q^T J Q) via a K=1
    matmul) -> NegIF2 state in PSUM (A-layout)
    conv branch: conv1 folded into conv2 center tap + all BN folds; 9
    shifted matmuls over a zero-padded spike tile into the output PSUM
    output = conv + identity (I-matmul) + 0.5*inv2*s~2 (diag matmul),
    evacuated by one ACT op that adds all remaining per-channel biases.
"""
import os
import sys
sys.path.insert(0, '/opt/trn_rl_repo')
sys.path.insert(0, '/root/.axon_site/_ro/trn_rl_repo')

import numpy as np

# ---------------------------------------------------------------- tile fix
import bass_rust
from concourse import bass, mybir
import concourse.tile as tile_mod
from concourse.tile import TileContext
from concourse.vector_clock import ScopedClock
from concourse.bass_utils import run_bass_kernel_spmd

MAX_WAITS = 1


def _patched_drain_and_barrier(self, tick_clock, wait_clock):
    drain_inst = self.nc.sync.drain()
    wait_clock.add_sem_waits(
        drain_inst.ins, ScopedClock({None: tick_clock.global_clock})
    )
    si = drain_inst.ins.sync_info
    if si is not None and si.on_wait and len(si.on_wait) > MAX_WAITS:
        waits = list(si.on_wait)
        si.on_wait = waits[:MAX_WAITS]
        for i in range(MAX_WAITS, len(waits), MAX_WAITS):
            nop = self.nc.sync.nop(nofuse=True, hint="wait_spill")
            nop.ins.sync_info = bass_rust.SyncInfo(
                on_wait=waits[i:i + MAX_WAITS], on_update=[]
            )
    self.nc.all_engine_barrier()
    assert self.sems is not None
    popped = self.nc._tile_sem_poison_stack.pop()
    assert popped is self._sem_poison
    self.nc.clear_and_free_semaphores(list(self.sems.allocated().values()))
    self.nc.all_engine_barrier()


tile_mod.TileContext._drain_and_barrier = _patched_drain_and_barrier


def _split_excess_waits(nc):
    """Walrus in this build rejects >1 sync wait per instruction; spill
    excess waits onto same-engine nops inserted before the instruction."""
    n_split = 0
    for bb in nc.main_func.blocks:
        insts = list(bb.instructions)
        out, changed = [], False
        for ins in insts:
            si = ins.sync_info
            if si is not None and si.on_wait and len(si.on_wait) > MAX_WAITS:
                waits = list(si.on_wait)
                si.on_wait = waits[-MAX_WAITS:]
                for i in range(0, len(waits) - MAX_WAITS, MAX_WAITS):
                    nop = mybir.InstNoOp(name=f"{ins.name}_wsp{i}", ins=[],
                                         outs=[])
                    nop.engine = ins.engine
                    nop.sync_info = bass_rust.SyncInfo(
                        on_wait=waits[i:i + MAX_WAITS], on_update=[])
                    out.append(nop)
                    n_split += 1
                changed = True
            out.append(ins)
        if changed:
            try:
                bb.instructions = out
            except Exception:
                lst = bb.instructions
                lst.clear()
                lst.extend(out)
    return n_split


# ---------------------------------------------------------------- consts
EPS = 1e-5
T, Bb, C, Hh, Ww = 4, 8, 512, 32, 32
NQ, PQ, HW = 4, 128, 1024
NCORES = 8
F32 = mybir.dt.float32
F16 = mybir.dt.float16
U8 = mybir.dt.uint8
ALU = mybir.AluOpType
AF = mybir.ActivationFunctionType
TAPS = [(dy, dx) for dy in (-1, 0, 1) for dx in (-1, 0, 1)]


def _haar_matrix(n):
    h = np.array([[1.0]])
    while h.shape[0] < n:
        top = np.kron(h, [1.0, 1.0])
        bot = np.kron(np.eye(h.shape[0]), [1.0, -1.0])
        h = np.concatenate([top, bot], axis=0) / np.sqrt(2.0)
    return h


def _bd4(block_fn):
    L = np.zeros((128, 128))
    for cb in range(4):
        L[32 * cb:32 * cb + 32, 32 * cb:32 * cb + 32] = block_fn(cb)
    return L


def _host_consts(inputs):
    hw_ = np.asarray(inputs['haar_weight'], np.float64)
    w1 = np.asarray(inputs['conv1_w'], np.float64)[:, :, 0, 0]
    b1 = np.asarray(inputs['conv1_b'], np.float64)
    w2 = np.asarray(inputs['conv2_w'], np.float64)
    b2 = np.asarray(inputs['conv2_b'], np.float64)
    bnw = np.asarray(inputs['bn_weight'], np.float64)
    bnb = np.asarray(inputs['bn_bias'], np.float64)
    bnm = np.asarray(inputs['bn_mean'], np.float64)
    bnv = np.asarray(inputs['bn_var'], np.float64)
    inv = bnw / np.sqrt(bnv + EPS)
    bbias = bnb - bnm * inv

    Q = _haar_matrix(32)
    P0flat = (Q.T @ np.ones((32, 32)) @ Q).reshape(HW)

    d = {}
    d['lfwdT'] = _bd4(lambda cb: Q.T).astype(np.float16)
    d['linvT'] = _bd4(lambda cb: Q).astype(np.float16)
    d['negIT'] = (-0.5 * np.eye(128)).astype(np.float16)
    d['eyeT'] = (2.0 * np.eye(128)).astype(np.float16)
    d['p0row'] = P0flat.reshape(1, HW).astype(np.float16)

    mixT = np.zeros((NQ, 128, 128))
    convT = np.zeros((NQ, 9, 128, 128))
    a2dT = np.zeros((NQ, 128, 128))
    beta1 = np.zeros((NQ, 1, 128))
    beta_all = np.zeros((NQ, 128, 1), np.float32)
    inv0c = np.zeros((NQ, 128, 32))
    thr2 = np.zeros((NQ, 128, 32))
    negB = np.zeros((NQ, 128, 32))

    for q in range(NQ):
        cidx = np.arange(128 * q, 128 * q + 128)

        def cv(k):
            return inv[k][cidx], bbias[k][cidx]

        inv0q, bias0q = cv(0)
        inv1q, bias1q = cv(1)
        inv2q, bias2q = cv(2)
        inv3q, bias3q = cv(3)
        inv4q, bias4q = cv(4)

        mixT[q] = _bd4(
            lambda cb: hw_[4 * q + cb] * inv1q.reshape(4, 32)[cb][None, :])
        for ti, (dy, dx) in enumerate(TAPS):
            def cb_blk(cb, dy=dy, dx=dx):
                m = w2[:, :, dy + 1, dx + 1].T * \
                    inv4q.reshape(4, 32)[cb][None, :]
                if dy == 0 and dx == 0:
                    m = m + w1.T * inv3q.reshape(4, 32)[cb][None, :]
                return m
            convT[q, ti] = _bd4(cb_blk)
        a2dT[q] = np.diag(inv2q / 2.0)
        beta1[q, 0] = bias1q
        beta_all[q, :, 0] = (inv4q * np.tile(b2, 16)[cidx] + bias4q
                             + inv3q * np.tile(b1, 16)[cidx] + bias3q
                             + bias2q)

        def cbc(v):
            return np.repeat(v.reshape(4, 32), 32, axis=0)

        inv0c[q] = cbc(inv0q) / 4.0
        thr2[q] = 1.0 - cbc(bias0q)
        negB[q] = -cbc(bias0q)

    d['mixT'] = mixT.astype(np.float16)
    d['convT'] = convT.astype(np.float16)
    d['a2dT'] = a2dT.astype(np.float16)
    d['beta1'] = beta1.astype(np.float16)
    d['beta_all'] = beta_all.astype(np.float32)
    d['inv0c'] = inv0c.astype(np.float16)
    d['thr2'] = thr2.astype(np.float16)
    d['negB'] = negB.astype(np.float16)
    return d


CONST_SPECS = [
    ('lfwdT', [128, 128], F16), ('linvT', [128, 128], F16),
    ('negIT', [128, 128], F16), ('eyeT', [128, 128], F16),
    ('p0row', [1, HW], F16),
    ('mixT', [NQ, 128, 128], F16), ('convT', [NQ, 9, 128, 128], F16),
    ('a2dT', [NQ, 128, 128], F16), ('beta1', [NQ, 1, 128], F16),
    ('beta_all', [NQ, 128, 1], F32),
    ('inv0c', [NQ, 128, 32], F16), ('thr2', [NQ, 128, 32], F16),
    ('negB', [NQ, 128, 32], F16),
]


def _build_program():
    nc = bass.Bass("TRN2", target_bir_lowering=False, debug=False)
    x32d = nc.declare_dram_parameter("x32h", [NQ, T, 128, HW], F32,
                                     isOutput=False)
    x16d = nc.declare_dram_parameter("x16h", [NQ, T, 128, HW], F16,
                                     isOutput=False)
    cdram = {}
    for name, shape, dt in CONST_SPECS:
        cdram[name] = nc.declare_dram_parameter(name, shape, dt,
                                                isOutput=False)
    outd = nc.declare_dram_parameter("out", [T, C, HW], F32, isOutput=True)

    with TileContext(nc) as tc:
        with (
            tc.tile_pool(name="consts", bufs=1) as cpool,
            tc.tile_pool(name="xp", bufs=3) as xpool,
            tc.tile_pool(name="state", bufs=2) as spool,
            tc.tile_pool(name="work", bufs=3) as wpool,
            tc.tile_pool(name="psV1", bufs=1, space="PSUM") as psV1,
            tc.tile_pool(name="psV2", bufs=1, space="PSUM") as psV2,
            tc.tile_pool(name="pstr", bufs=2, space="PSUM") as pstr,
        ):
            # ---- load constants into SBUF once ----
            ct = {}
            for name, shape, dt in CONST_SPECS:
                flat = [int(np.prod(shape[:-1])), shape[-1]]
                if len(shape) > 2 and shape[-2] in (128, 1):
                    # keep [.., P, F] structure: partition dim = shape[-2]
                    tile = cpool.tile([shape[-2],
                                       int(np.prod(shape) // shape[-2])], dt,
                                      tag=f"c_{name}")
                    # DMA each leading index into columns
                    n_lead = int(np.prod(shape[:-2]))
                    fs = shape[-1]
                    src = cdram[name].ap().rearrange(
                        "... p f -> p (... f)") if False else None
                    # simpler: per-leading-slice DMA
                    dview = cdram[name].ap()
                    dflat = dview.rearrange(
                        "a p f -> a p f") if len(shape) == 3 else (
                        dview.rearrange("a b p f -> (a b) p f")
                        if len(shape) == 4 else dview)
                    if len(shape) == 2:
                        nc.sync.dma_start(tile[:, :], dview)
                    else:
                        for li in range(n_lead):
                            nc.sync.dma_start(
                                tile[:, li * fs:(li + 1) * fs], dflat[li])
                    ct[name] = (tile, shape)
                else:
                    tile = cpool.tile(flat, dt, tag=f"c_{name}")
                    nc.sync.dma_start(tile[:, :], cdram[name].ap().rearrange(
                        "... f -> (...) f") if len(shape) > 2 else
                        cdram[name].ap())
                    ct[name] = (tile, shape)

            def cslice(name, *lead):
                tile, shape = ct[name]
                fs = shape[-1]
                n_lead_dims = len(shape) - 2
                li = 0
                for k in range(n_lead_dims):
                    li = li * shape[k] + lead[k]
                return tile[:, li * fs:(li + 1) * fs]

            one_m1 = cpool.tile([128, 1], F32, tag="bm1")
            one_p1 = cpool.tile([128, 1], F32, tag="bp1")
            zero32 = cpool.tile([128, 1], F32, tag="z32")
            nc.vector.memset(one_m1[:, :], -1.0)
            nc.vector.memset(one_p1[:, :], 1.0)
            nc.vector.memset(zero32[:, :], 0.0)

            for q in range(NQ):
                u1 = spool.tile([128, HW], F32, tag="u1")
                u2h = spool.tile([128, HW], F16, tag="u2h")
                nc.gpsimd.memset(u1[:, :], 0.0)
                nc.gpsimd.tensor_copy(
                    u2h[:, :].rearrange("p (a b) -> p a b", a=32, b=32),
                    cslice('negB', q).rearrange(
                        "p (o a) -> p o a", o=1, a=32).rearrange(
                        "p o a -> p a o").broadcast_to((128, 32, 32)))
                V1 = psV1.tile([128, HW], F32, tag="V1")
                V2 = psV2.tile([128, HW], F32, tag="V2")

                for t in range(T):
                    xt32 = xpool.tile([128, HW], F32, tag="xt32")
                    nc.sync.dma_start(xt32[:, :], x32d.ap()[q, t])
                    xt = xpool.tile([128, HW], F16, tag="xt")
                    nc.sync.dma_start(xt[:, :], x16d.ap()[q, t])

                    # ---- LIF1 (A-layout): u1 = 0.5*u1 + x/2 ----
                    nc.vector.scalar_tensor_tensor(
                        u1[:, :], u1[:, :], 0.5, xt32[:, :],
                        ALU.mult, ALU.add)
                    sp = wpool.tile([128, 34 * 34], F16, tag="sp")
                    nc.gpsimd.memset(sp[:, :], 0.0)
                    sp3 = sp[:, :].rearrange("p (h w) -> p h w", h=34, w=34)
                    nc.gpsimd.tensor_scalar(
                        sp3[:, 1:33, 1:33],
                        u1[:, :].rearrange("p (h w) -> p h w", h=32, w=32),
                        1.0, None, ALU.is_ge)
                    m1 = wpool.tile([128, HW], U8, tag="m1")
                    nc.vector.tensor_scalar(m1[:, :], u1[:, :], 1.0, None,
                                            ALU.is_ge)
                    nc.vector.copy_predicated(
                        u1[:, :], m1[:, :],
                        zero32[:, :].broadcast_to((128, HW)))

                    # ---- ST1: padded A -> B ----
                    sB = wpool.tile([128, HW], F16, tag="sB")
                    spT = sp[:, :].rearrange("p (h w) -> p w h", h=34, w=34)
                    nc.vector.transpose(
                        sB[:, :].rearrange("p (h w) -> p w h", h=32, w=32),
                        spT[:, 1:33, 1:33])

                    # ---- fwd row MM ----
                    rowOut = pstr.tile([128, HW], F32, tag="tr")
                    lfwdT = cslice('lfwdT')
                    for h_ in (0, 512):
                        nc.tensor.matmul(rowOut[:, h_:h_ + 512], lfwdT,
                                         sB[:, h_:h_ + 512], start=True,
                                         stop=True)
                    # ---- ST2: ACT evac to f16, then DVE transpose ----
                    rowS = wpool.tile([128, HW], F16, tag="rowS")
                    nc.scalar.copy(rowS[:, :], rowOut[:, :])
                    C2 = wpool.tile([128, HW], F16, tag="C2")
                    nc.vector.transpose(C2[:, :], rowS[:, :])
                    # ---- fwd col MM: V1 accumulate ----
                    for h_ in (0, 512):
                        nc.tensor.matmul(V1[:, h_:h_ + 512], lfwdT,
                                         C2[:, h_:h_ + 512], start=(t == 0),
                                         stop=(t == T - 1),
                                         skip_group_check=True)

                    # ---- negif1 spikes ----
                    g1 = wpool.tile([128, HW], F16, tag="g1")
                    g2 = wpool.tile([128, HW], F16, tag="g2")
                    nc.scalar.activation(g1[:, :], V1[:, :], AF.Sign,
                                         bias=one_m1[:, :])
                    nc.scalar.activation(g2[:, :], V1[:, :], AF.Sign,
                                         bias=one_p1[:, :])
                    st1 = wpool.tile([128, HW], F16, tag="st1")
                    nc.gpsimd.tensor_tensor(st1[:, :], g1[:, :], g2[:, :],
                                            ALU.add)
                    negIT = cslice('negIT')
                    for h_ in (0, 512):
                        nc.tensor.matmul(V1[:, h_:h_ + 512], negIT,
                                         st1[:, h_:h_ + 512], start=False,
                                         stop=False, skip_group_check=True)

                    # ---- bn0 + LIF2 (C-layout, broadcast params) ----
                    v3 = lambda ap: ap.rearrange("p (a b) -> p a b", a=32,
                                                 b=32)
                    bc = lambda name: cslice(name, q).rearrange(
                        "p (o a) -> p o a", o=1, a=32).rearrange(
                        "p o a -> p a o").broadcast_to((128, 32, 32))
                    pprod = wpool.tile([128, HW], F16, tag="pprod")
                    nc.gpsimd.tensor_tensor(v3(pprod[:, :]), v3(st1[:, :]),
                                            bc('inv0c'), ALU.mult)
                    nc.vector.scalar_tensor_tensor(
                        u2h[:, :], u2h[:, :], 0.5, pprod[:, :],
                        ALU.mult, ALU.add)
                    s2 = wpool.tile([128, HW], F16, tag="s2")
                    nc.vector.tensor_tensor(v3(s2[:, :]), v3(u2h[:, :]),
                                            bc('thr2'), ALU.is_ge)
                    m2 = wpool.tile([128, HW], U8, tag="m2")
                    nc.vector.tensor_tensor(v3(m2[:, :]), v3(u2h[:, :]),
                                            bc('thr2'), ALU.is_ge)
                    nc.vector.copy_predicated(v3(u2h[:, :]), v3(m2[:, :]),
                                              bc('negB'))

                    # ---- inverse Haar from C ----
                    linvT = cslice('linvT')
                    Z = pstr.tile([128, HW], F32, tag="tr")
                    for h_ in (0, 512):
                        nc.tensor.matmul(Z[:, h_:h_ + 512], linvT,
                                         s2[:, h_:h_ + 512], start=True,
                                         stop=True)
                    Zs = wpool.tile([128, HW], F16, tag="Zs")
                    nc.scalar.copy(Zs[:, :], Z[:, :])
                    Z2 = wpool.tile([128, HW], F16, tag="Z2")
                    nc.vector.transpose(Z2[:, :], Zs[:, :])
                    W2 = pstr.tile([128, HW], F32, tag="tr")
                    for h_ in (0, 512):
                        nc.tensor.matmul(W2[:, h_:h_ + 512], linvT,
                                         Z2[:, h_:h_ + 512], start=True,
                                         stop=True)
                    W2s = wpool.tile([128, HW], F16, tag="W2s")
                    nc.scalar.copy(W2s[:, :], W2[:, :])
                    haarA = wpool.tile([128, HW], F16, tag="haarA")
                    nc.vector.transpose(
                        haarA[:, :].rearrange("p (a b) -> p b a", a=32, b=32),
                        W2s[:, :].rearrange("p (cc b) -> p b cc", cc=32, b=32))

                    # ---- mix' + P0 bias -> V2 ----
                    mixT = cslice('mixT', q)
                    for h_ in (0, 512):
                        nc.tensor.matmul(V2[:, h_:h_ + 512], mixT,
                                         haarA[:, h_:h_ + 512],
                                         start=(t == 0), stop=(t == T - 1),
                                         skip_group_check=True)
                    beta1 = ct['beta1'][0][0:1,
                                           q * 128:(q + 1) * 128]
                    p0row = ct['p0row'][0]
                    for h_ in (0, 512):
                        nc.tensor.matmul(V2[:, h_:h_ + 512], beta1,
                                         p0row[0:1, h_:h_ + 512], start=False,
                                         stop=False, skip_group_check=True)

                    # ---- negif2 ----
                    g1b = wpool.tile([128, HW], F16, tag="g1b")
                    g2b = wpool.tile([128, HW], F16, tag="g2b")
                    nc.scalar.activation(g1b[:, :], V2[:, :], AF.Sign,
                                         bias=one_m1[:, :])
                    nc.scalar.activation(g2b[:, :], V2[:, :], AF.Sign,
                                         bias=one_p1[:, :])
                    st2 = wpool.tile([128, HW], F16, tag="st2")
                    nc.gpsimd.tensor_tensor(st2[:, :], g1b[:, :], g2b[:, :],
                                            ALU.add)
                    for h_ in (0, 512):
                        nc.tensor.matmul(V2[:, h_:h_ + 512], negIT,
                                         st2[:, h_:h_ + 512], start=False,
                                         stop=False, skip_group_check=True)

                    # ---- OUT psum: conv + identity + haar term ----
                    OUT = pstr.tile([128, HW], F32, tag="tr")
                    for ti in range(9):
                        dy, dx = TAPS[ti]
                        cT = cslice('convT', q, ti)
                        off_h = 1 + dy
                        off_w = 1 + dx
                        rhs = sp3[:, off_h:off_h + 32, off_w:off_w + 32]
                        nc.tensor.matmul(OUT[:, 0:512], cT, rhs[:, 0:16, :],
                                         start=(ti == 0), stop=False,
                                         skip_group_check=True)
                        nc.tensor.matmul(OUT[:, 512:1024], cT,
                                         rhs[:, 16:32, :],
                                         start=(ti == 0), stop=False,
                                         skip_group_check=True)
                    eyeT = cslice('eyeT')
                    a2dT = cslice('a2dT', q)
                    for h_ in (0, 512):
                        nc.tensor.matmul(OUT[:, h_:h_ + 512], eyeT,
                                         xt[:, h_:h_ + 512], start=False,
                                         stop=False, skip_group_check=True)
                        nc.tensor.matmul(OUT[:, h_:h_ + 512], a2dT,
                                         st2[:, h_:h_ + 512], start=False,
                                         stop=True, skip_group_check=True)

                    # ---- evacuate with bias, DMA out ----
                    osb = wpool.tile([128, HW], F32, tag="osb")
                    nc.scalar.activation(osb[:, :], OUT[:, :], AF.Identity,
                                         bias=cslice('beta_all', q))
                    nc.sync.dma_start(
                        outd.ap()[t, q * 128:(q + 1) * 128, :], osb[:, :])

    _split_excess_waits(nc)
    return nc


_NC_CACHE = None


def _get_nc():
    global _NC_CACHE
    if _NC_CACHE is None:
        _NC_CACHE = _build_program()
    return _NC_CACHE


def kernel(**inputs):
    x = np.asarray(inputs['x'], np.float32)          # [T, B, C, H, W]
    consts = _host_consts(inputs)

    in_maps = []
    for b in range(NCORES):
        xb = (0.5 * x[:, b]).reshape(T, NQ, 128, HW).transpose(1, 0, 2, 3)
        xb = np.ascontiguousarray(xb)
        m = {'x32h': xb.astype(np.float32),
             'x16h': xb.astype(np.float16)}
        m.update({k: consts[k] for k, _, _ in CONST_SPECS})
        in_maps.append(m)

    nc = _get_nc()
    res = run_bass_kernel_spmd(nc, in_maps, list(range(NCORES))).results
    out = np.stack([res[b]['out'] for b in range(NCORES)], axis=1)
    return out.reshape(T, Bb, C, Hh, Ww).astype(np.float32)


# revision 6
# speedup vs baseline: 1.4141x; 1.4141x over previous
"""FATM (wavelet spiking module) Trainium2 Bass kernel.

Data-parallel over B across 8 NeuronCores (B=8 -> 1 sample/core).

Per-core pipeline (layout algebra validated in numpy vs the reference):
  chunk-serial over 4 chunks of 128 channels, t-serial over T=4:
    LIF1 (A-layout, fp32 state, scalar_tensor_tensor decay update)
    spikes written twice: zero-padded tile (conv taps) + flat tile;
    all four 32x32-block stream-transposes run on flat unit-stride APs --
    the free-dim permutes they would otherwise need are absorbed into the
    ACT PSUM-evacuation copies (strided output APs are free there).
    fwd Haar: col-transform matmul, evac+permute, ST, row-transform into
    the NegIF1 PSUM accumulator. NegIF spikes: two ACT Sign ops
    (s~ = sign(v-1)+sign(v+1) = 2s), soft reset via -0.5*I matmul feedback.
    bn0+LIF2 in transformed layout via broadcast param tiles (bn0 bias
    folded into thresholds/reset values; fixed-point offset form keeps
    fp16 ranges O(1)). Inverse Haar, then channel-mix (bn1 scale folded in;
    bn1 bias enters as bias1[c]*(Q^T J Q) via a K=1 matmul) -> NegIF2 PSUM
    (A-layout). Conv branch: conv1 folded into conv2 center tap + BN folds;
    9 shifted matmuls over the padded spike tile into the output PSUM,
    plus 0.5*inv2*s~2 (diag matmul) and all biases (K=1 ones matmul).
    Final: out = 2*(x/2) + OUT_psum via one DVE scalar_tensor_tensor.
  Spike reset masks are fp16 spike tiles bitcast to uint16.
"""
import os
import sys
sys.path.insert(0, '/opt/trn_rl_repo')
sys.path.insert(0, '/root/.axon_site/_ro/trn_rl_repo')

import numpy as np

import bass_rust
from concourse import bass, mybir
import concourse.tile as tile_mod
from concourse.tile import TileContext
from concourse.vector_clock import ScopedClock
from concourse.bass_utils import run_bass_kernel_spmd

# ------------------------------------------------------------- walrus fix
MAX_WAITS = 1


def _patched_drain_and_barrier(self, tick_clock, wait_clock):
    drain_inst = self.nc.sync.drain()
    wait_clock.add_sem_waits(
        drain_inst.ins, ScopedClock({None: tick_clock.global_clock})
    )
    si = drain_inst.ins.sync_info
    if si is not None and si.on_wait and len(si.on_wait) > MAX_WAITS:
        waits = list(si.on_wait)
        si.on_wait = waits[:MAX_WAITS]
        for i in range(MAX_WAITS, len(waits), MAX_WAITS):
            nop = self.nc.sync.nop(nofuse=True, hint="wait_spill")
            nop.ins.sync_info = bass_rust.SyncInfo(
                on_wait=waits[i:i + MAX_WAITS], on_update=[]
            )
    self.nc.all_engine_barrier()
    assert self.sems is not None
    popped = self.nc._tile_sem_poison_stack.pop()
    assert popped is self._sem_poison
    self.nc.clear_and_free_semaphores(list(self.sems.allocated().values()))
    self.nc.all_engine_barrier()


tile_mod.TileContext._drain_and_barrier = _patched_drain_and_barrier


def _split_excess_waits(nc):
    """This walrus build rejects >1 sync wait per instruction; spill excess
    waits onto same-engine nops inserted before the instruction."""
    n_split = 0
    for bb in nc.main_func.blocks:
        insts = list(bb.instructions)
        out, changed = [], False
        for ins in insts:
            si = ins.sync_info
            if si is not None and si.on_wait and len(si.on_wait) > MAX_WAITS:
                waits = list(si.on_wait)
                si.on_wait = waits[-MAX_WAITS:]
                for i in range(0, len(waits) - MAX_WAITS, MAX_WAITS):
                    nop = mybir.InstNoOp(name=f"{ins.name}_wsp{i}", ins=[],
                                         outs=[])
                    nop.engine = ins.engine
                    nop.sync_info = bass_rust.SyncInfo(
                        on_wait=waits[i:i + MAX_WAITS], on_update=[])
                    out.append(nop)
                    n_split += 1
                changed = True
            out.append(ins)
        if changed:
            try:
                bb.instructions = out
            except Exception:
                lst = bb.instructions
                lst.clear()
                lst.extend(out)
    return n_split


# ---------------------------------------------------------------- consts
EPS = 1e-5
T, Bb, C, Hh, Ww = 4, 8, 512, 32, 32
NQ, HW = 4, 1024
NCORES = 8
F32 = mybir.dt.float32
F16 = mybir.dt.float16
U16 = mybir.dt.uint16
ALU = mybir.AluOpType
AF = mybir.ActivationFunctionType
TAPS = [(dy, dx) for dy in (-1, 0, 1) for dx in (-1, 0, 1)]

# fp16 [128, x] consts packed into one DRAM array (order defines offsets)
PACK128 = [
    ('lfwdT', 128), ('linvT', 128), ('negIT', 128),
    ('mixT', NQ * 128), ('convT', NQ * 9 * 128), ('a2dT', NQ * 128),
    ('inv0c', NQ * 32), ('thr2', NQ * 32), ('negB', NQ * 32),
]
PACK1 = [('p0row', HW), ('ones', HW), ('beta1', NQ * 128),
         ('betaA', NQ * 128)]


def _haar_matrix(n):
    h = np.array([[1.0]])
    while h.shape[0] < n:
        top = np.kron(h, [1.0, 1.0])
        bot = np.kron(np.eye(h.shape[0]), [1.0, -1.0])
        h = np.concatenate([top, bot], axis=0) / np.sqrt(2.0)
    return h


def _bd4(block_fn):
    L = np.zeros((128, 128))
    for cb in range(4):
        L[32 * cb:32 * cb + 32, 32 * cb:32 * cb + 32] = block_fn(cb)
    return L


def _host_consts(inputs):
    hw_ = np.asarray(inputs['haar_weight'], np.float64)
    w1 = np.asarray(inputs['conv1_w'], np.float64)[:, :, 0, 0]
    b1 = np.asarray(inputs['conv1_b'], np.float64)
    w2 = np.asarray(inputs['conv2_w'], np.float64)
    b2 = np.asarray(inputs['conv2_b'], np.float64)
    bnw = np.asarray(inputs['bn_weight'], np.float64)
    bnb = np.asarray(inputs['bn_bias'], np.float64)
    bnm = np.asarray(inputs['bn_mean'], np.float64)
    bnv = np.asarray(inputs['bn_var'], np.float64)
    inv = bnw / np.sqrt(bnv + EPS)
    bbias = bnb - bnm * inv

    Q = _haar_matrix(32)
    P0flat = (Q.T @ np.ones((32, 32)) @ Q).reshape(HW)

    d = {}
    d['lfwdT'] = _bd4(lambda cb: Q.T)        # [p=(cb,w), m=(cb,l)] = Q[l,w]
    d['linvT'] = _bd4(lambda cb: Q)          # [p=(cb,i), m=(cb,a)] = Q[i,a]
    d['negIT'] = -0.5 * np.eye(128)
    d['p0row'] = P0flat.reshape(1, HW)
    d['ones'] = np.ones((1, HW))

    mixT = np.zeros((NQ, 128, 128))
    convT = np.zeros((NQ, 9, 128, 128))
    a2dT = np.zeros((NQ, 128, 128))
    beta1 = np.zeros((NQ, 128))
    betaA = np.zeros((NQ, 128))
    inv0c = np.zeros((NQ, 128, 32))
    thr2 = np.zeros((NQ, 128, 32))
    negB = np.zeros((NQ, 128, 32))

    for q in range(NQ):
        cidx = np.arange(128 * q, 128 * q + 128)
        inv0q, bias0q = inv[0][cidx], bbias[0][cidx]
        inv1q, bias1q = inv[1][cidx], bbias[1][cidx]
        inv2q, bias2q = inv[2][cidx], bbias[2][cidx]
        inv3q, bias3q = inv[3][cidx], bbias[3][cidx]
        inv4q, bias4q = inv[4][cidx], bbias[4][cidx]

        mixT[q] = _bd4(
            lambda cb: hw_[4 * q + cb] * inv1q.reshape(4, 32)[cb][None, :])
        for ti, (dy, dx) in enumerate(TAPS):
            def cb_blk(cb, dy=dy, dx=dx):
                m = w2[:, :, dy + 1, dx + 1].T * \
                    inv4q.reshape(4, 32)[cb][None, :]
                if dy == 0 and dx == 0:
                    m = m + w1.T * inv3q.reshape(4, 32)[cb][None, :]
                return m
            convT[q, ti] = _bd4(cb_blk)
        a2dT[q] = np.diag(inv2q / 2.0)
        beta1[q] = bias1q
        betaA[q] = (inv4q * np.tile(b2, 16)[cidx] + bias4q
                    + inv3q * np.tile(b1, 16)[cidx] + bias3q + bias2q)

        def cbc(v):
            return np.repeat(v.reshape(4, 32), 32, axis=0)

        inv0c[q] = cbc(inv0q) / 4.0
        thr2[q] = 1.0 - cbc(bias0q)
        negB[q] = -cbc(bias0q)

    d['mixT'] = mixT.transpose(1, 0, 2).reshape(128, NQ * 128)
    d['convT'] = convT.transpose(2, 0, 1, 3).reshape(128, NQ * 9 * 128)
    d['a2dT'] = a2dT.transpose(1, 0, 2).reshape(128, NQ * 128)
    d['inv0c'] = inv0c.transpose(1, 0, 2).reshape(128, NQ * 32)
    d['thr2'] = thr2.transpose(1, 0, 2).reshape(128, NQ * 32)
    d['negB'] = negB.transpose(1, 0, 2).reshape(128, NQ * 32)
    d['beta1'] = beta1.reshape(1, NQ * 128)
    d['betaA'] = betaA.reshape(1, NQ * 128)

    c128 = np.concatenate(
        [np.asarray(d[n]).reshape(128, w) for n, w in PACK128],
        axis=1).astype(np.float16)
    c1 = np.concatenate(
        [np.asarray(d[n]).reshape(1, w) for n, w in PACK1],
        axis=1).astype(np.float16)
    return np.ascontiguousarray(c128), np.ascontiguousarray(c1)


P128_W = sum(w for _, w in PACK128)
P1_W = sum(w for _, w in PACK1)
P128_OFF = {}
_o = 0
for _n, _w in PACK128:
    P128_OFF[_n] = (_o, _w)
    _o += _w
P1_OFF = {}
_o = 0
for _n, _w in PACK1:
    P1_OFF[_n] = (_o, _w)
    _o += _w


def _build_program():
    nc = bass.Bass("TRN2", target_bir_lowering=False, debug=False)
    x32d = nc.declare_dram_parameter("x32h", [NQ, 128, T * HW], F32,
                                     isOutput=False)
    c128d = nc.declare_dram_parameter("c128", [128, P128_W], F16,
                                      isOutput=False)
    c1d = nc.declare_dram_parameter("c1", [1, P1_W], F16, isOutput=False)
    outd = nc.declare_dram_parameter("out", [NQ, 128, T * HW], F32,
                                     isOutput=True)

    with TileContext(nc) as tc:
        with (
            tc.tile_pool(name="consts", bufs=1) as cpool,
            tc.tile_pool(name="xq", bufs=2) as xqpool,
            tc.tile_pool(name="ost", bufs=2) as ostpool,
            tc.tile_pool(name="state", bufs=2) as spool,
            tc.tile_pool(name="spp", bufs=2) as sppool,
            tc.tile_pool(name="work", bufs=3) as wpool,
            tc.tile_pool(name="psV1", bufs=1, space="PSUM") as psV1,
            tc.tile_pool(name="psV2", bufs=1, space="PSUM") as psV2,
            tc.tile_pool(name="pstr", bufs=2, space="PSUM") as pstr,
        ):
            c128 = cpool.tile([128, P128_W], F16, tag="c128")
            nc.sync.dma_start(c128[:, :], c128d.ap())
            c1 = cpool.tile([1, P1_W], F16, tag="c1")
            nc.sync.dma_start(c1[:, :], c1d.ap())
            one_m1 = cpool.tile([128, 1], F32, tag="bm1")
            one_p1 = cpool.tile([128, 1], F32, tag="bp1")
            zero32 = cpool.tile([128, 1], F32, tag="z32")
            nc.vector.memset(one_m1[:, :], -1.0)
            nc.vector.memset(one_p1[:, :], 1.0)
            nc.vector.memset(zero32[:, :], 0.0)

            def c128s(name, idx=0, w=128):
                off, _ = P128_OFF[name]
                return c128[:, off + idx * w: off + (idx + 1) * w]

            def c1s(name, idx=0, w=None):
                off, tot = P1_OFF[name]
                if w is None:
                    w = tot
                return c1[0:1, off + idx * w: off + (idx + 1) * w]

            def bcast(name, q):
                # [128, 32] chunk-param -> [128, 32, 32] stride-0 inner AP
                ap = c128s(name, q, 32)
                return ap.rearrange("p (o a) -> p o a", o=1, a=32).rearrange(
                    "p o a -> p a o").broadcast_to((128, 32, 32))

            v3 = lambda ap: ap.rearrange("p (a b) -> p a b", a=32, b=32)

            for q in range(NQ):
                xq = xqpool.tile([128, T * HW], F32, tag="xq")
                nc.sync.dma_start(xq[:, :], x32d.ap()[q])
                ostage = ostpool.tile([128, T * HW], F32, tag="ost")

                u1 = spool.tile([128, HW], F32, tag="u1")
                u2h = spool.tile([128, HW], F16, tag="u2h")
                nc.gpsimd.memset(u1[:, :], 0.0)
                nc.gpsimd.tensor_copy(v3(u2h[:, :]), bcast('negB', q))
                V1 = psV1.tile([128, HW], F32, tag="V1")
                V2 = psV2.tile([128, HW], F32, tag="V2")

                for t in range(T):
                    xt32 = xq[:, t * HW:(t + 1) * HW]

                    # ---- LIF1: u1 = 0.5*u1 + x/2; spike at 1.0 ----
                    nc.vector.scalar_tensor_tensor(
                        u1[:, :], u1[:, :], 0.5, xt32, ALU.mult, ALU.add)
                    sp = sppool.tile([128, 34 * 34], F16, tag="sp")
                    if q == 0 and t < 2:
                        nc.gpsimd.memset(sp[:, :], 0.0)
                    sp3 = sp[:, :].rearrange("p (h w) -> p h w", h=34, w=34)
                    nc.vector.tensor_scalar(
                        sp3[:, 1:33, 1:33],
                        u1[:, :].rearrange("p (h w) -> p h w", h=32, w=32),
                        1.0, None, ALU.is_ge)
                    sN = wpool.tile([128, HW], F16, tag="sN")
                    nc.vector.tensor_scalar(sN[:, :], u1[:, :], 1.0, None,
                                            ALU.is_ge)
                    nc.vector.copy_predicated(
                        u1[:, :], sN[:, :].bitcast(U16),
                        zero32[:, :].broadcast_to((128, HW)))

                    # ---- ST1 (flat): A -> D [(cb,w), 32h+cc] ----
                    sD = wpool.tile([128, HW], F16, tag="sD")
                    nc.vector.transpose(sD[:, :], sN[:, :])
                    # ---- fwd col MM: M1[(cb,l), 32h+cc] ----
                    M1 = pstr.tile([128, HW], F32, tag="tr")
                    lfwdT = c128s('lfwdT')
                    linvT = c128s('linvT')
                    negIT = c128s('negIT')
                    for h_ in (0, 512):
                        nc.tensor.matmul(M1[:, h_:h_ + 512], lfwdT,
                                         sD[:, h_:h_ + 512], start=True,
                                         stop=True)
                    # ---- evac with permute (32h+cc -> 32cc+h) ----
                    M1s = wpool.tile([128, HW], F16, tag="M1s")
                    nc.scalar.activation(
                        M1s[:, :].rearrange("p (cc h) -> p h cc", cc=32,
                                            h=32),
                        M1[:, :].rearrange("p (h cc) -> p h cc", h=32, cc=32),
                        AF.Copy)
                    # ---- ST2 (flat): -> B [(cb,h), 32cc+l] ----
                    sB = wpool.tile([128, HW], F16, tag="sB")
                    nc.vector.transpose(sB[:, :], M1s[:, :])
                    # ---- fwd row MM: V1 += [(cb,i), 32cc+l] ----
                    for h_ in (0, 512):
                        nc.tensor.matmul(V1[:, h_:h_ + 512], lfwdT,
                                         sB[:, h_:h_ + 512], start=(t == 0),
                                         stop=(t == T - 1),
                                         skip_group_check=True)

                    # ---- negif1 spikes ----
                    g1 = wpool.tile([128, HW], F16, tag="g1")
                    g2 = wpool.tile([128, HW], F16, tag="g2")
                    nc.scalar.activation(g1[:, :], V1[:, :], AF.Sign,
                                         bias=one_m1[:, :])
                    nc.scalar.activation(g2[:, :], V1[:, :], AF.Sign,
                                         bias=one_p1[:, :])
                    st1 = wpool.tile([128, HW], F16, tag="st1")
                    nc.gpsimd.tensor_tensor(st1[:, :], g1[:, :], g2[:, :],
                                            ALU.add)
                    for h_ in (0, 512):
                        nc.tensor.matmul(V1[:, h_:h_ + 512], negIT,
                                         st1[:, h_:h_ + 512], start=False,
                                         stop=False, skip_group_check=True)

                    # ---- bn0 + LIF2 (transformed layout) ----
                    pprod = wpool.tile([128, HW], F16, tag="pprod")
                    nc.gpsimd.tensor_tensor(v3(pprod[:, :]), v3(st1[:, :]),
                                            bcast('inv0c', q), ALU.mult)
                    nc.vector.scalar_tensor_tensor(
                        u2h[:, :], u2h[:, :], 0.5, pprod[:, :],
                        ALU.mult, ALU.add)
                    s2 = wpool.tile([128, HW], F16, tag="s2")
                    nc.vector.tensor_tensor(v3(s2[:, :]), v3(u2h[:, :]),
                                            bcast('thr2', q), ALU.is_ge)
                    nc.vector.copy_predicated(v3(u2h[:, :]),
                                              v3(s2[:, :].bitcast(U16)),
                                              bcast('negB', q))

                    # ---- inverse Haar: row-inv first ----
                    Z = pstr.tile([128, HW], F32, tag="tr")
                    for h_ in (0, 512):
                        nc.tensor.matmul(Z[:, h_:h_ + 512], linvT,
                                         s2[:, h_:h_ + 512], start=True,
                                         stop=True)
                    Zs = wpool.tile([128, HW], F16, tag="Zs")
                    nc.scalar.copy(Zs[:, :], Z[:, :])
                    ZT = wpool.tile([128, HW], F16, tag="ZT")
                    nc.vector.transpose(ZT[:, :], Zs[:, :])
                    W2 = pstr.tile([128, HW], F32, tag="tr")
                    for h_ in (0, 512):
                        nc.tensor.matmul(W2[:, h_:h_ + 512], linvT,
                                         ZT[:, h_:h_ + 512], start=True,
                                         stop=True)
                    Ws = wpool.tile([128, HW], F16, tag="Ws")
                    nc.scalar.activation(
                        Ws[:, :].rearrange("p (a cc) -> p cc a", a=32,
                                           cc=32),
                        W2[:, :].rearrange("p (cc a) -> p cc a", cc=32, a=32),
                        AF.Copy)
                    haarA = wpool.tile([128, HW], F16, tag="haarA")
                    nc.vector.transpose(haarA[:, :], Ws[:, :])

                    # ---- mix' + P0 bias -> V2 (A-layout) ----
                    for h_ in (0, 512):
                        nc.tensor.matmul(V2[:, h_:h_ + 512],
                                         c128s('mixT', q),
                                         haarA[:, h_:h_ + 512],
                                         start=(t == 0), stop=(t == T - 1),
                                         skip_group_check=True)
                    for h_ in (0, 512):
                        nc.tensor.matmul(V2[:, h_:h_ + 512],
                                         c1s('beta1', q, 128),
                                         c1s('p0row')[0:1, h_:h_ + 512],
                                         start=False, stop=False,
                                         skip_group_check=True)

                    # ---- negif2 ----
                    g1b = wpool.tile([128, HW], F16, tag="g1b")
                    g2b = wpool.tile([128, HW], F16, tag="g2b")
                    nc.scalar.activation(g1b[:, :], V2[:, :], AF.Sign,
                                         bias=one_m1[:, :])
                    nc.scalar.activation(g2b[:, :], V2[:, :], AF.Sign,
                                         bias=one_p1[:, :])
                    st2 = wpool.tile([128, HW], F16, tag="st2")
                    nc.gpsimd.tensor_tensor(st2[:, :], g1b[:, :], g2b[:, :],
                                            ALU.add)
                    for h_ in (0, 512):
                        nc.tensor.matmul(V2[:, h_:h_ + 512], negIT,
                                         st2[:, h_:h_ + 512], start=False,
                                         stop=False, skip_group_check=True)

                    # ---- OUT psum: conv + haar + biases ----
                    OUT = pstr.tile([128, HW], F32, tag="tr")
                    for ti in range(9):
                        dy, dx = TAPS[ti]
                        cT = c128s('convT', q * 9 + ti)
                        rhs = sp3[:, 1 + dy:33 + dy, 1 + dx:33 + dx]
                        nc.tensor.matmul(OUT[:, 0:512], cT, rhs[:, 0:16, :],
                                         start=(ti == 0), stop=False,
                                         skip_group_check=True)
                        nc.tensor.matmul(OUT[:, 512:1024], cT,
                                         rhs[:, 16:32, :],
                                         start=(ti == 0), stop=False,
                                         skip_group_check=True)
                    a2dT = c128s('a2dT', q)
                    for h_ in (0, 512):
                        nc.tensor.matmul(OUT[:, h_:h_ + 512], a2dT,
                                         st2[:, h_:h_ + 512], start=False,
                                         stop=False, skip_group_check=True)
                        nc.tensor.matmul(OUT[:, h_:h_ + 512],
                                         c1s('betaA', q, 128),
                                         c1s('ones')[0:1, h_:h_ + 512],
                                         start=False, stop=True,
                                         skip_group_check=True)

                    # ---- final: out = 2*(x/2) + OUT ----
                    nc.vector.scalar_tensor_tensor(
                        ostage[:, t * HW:(t + 1) * HW], xt32, 2.0,
                        OUT[:, :], ALU.mult, ALU.add)

                nc.sync.dma_start(outd.ap()[q], ostage[:, :])

    _split_excess_waits(nc)
    return nc


_NC_CACHE = None


def _get_nc():
    global _NC_CACHE
    if _NC_CACHE is None:
        _NC_CACHE = _build_program()
    return _NC_CACHE


def kernel(**inputs):
    x = np.asarray(inputs['x'], np.float32)          # [T, B, C, H, W]
    c128, c1 = _host_consts(inputs)

    in_maps = []
    for b in range(NCORES):
        # [T, C, HW] -> [NQ, 128, T*HW], halved for the LIF1 decay form
        xb = (0.5 * x[:, b]).reshape(T, NQ, 128, HW).transpose(1, 2, 0, 3)
        m = {'x32h': np.ascontiguousarray(xb).reshape(NQ, 128, T * HW)
             .astype(np.float32),
             'c128': c128, 'c1': c1}
        in_maps.append(m)

    nc = _get_nc()
    res = run_bass_kernel_spmd(nc, in_maps, list(range(NCORES))).results
    # out [NQ, 128, T*HW] -> [T, B, C, H, W]
    outs = []
    for b in range(NCORES):
        ob = res[b]['out'].reshape(NQ, 128, T, HW).transpose(2, 0, 1, 3)
        outs.append(ob.reshape(T, C, HW))
    out = np.stack(outs, axis=1)
    return out.reshape(T, Bb, C, Hh, Ww).astype(np.float32)


# revision 7
# speedup vs baseline: 1.5822x; 1.1189x over previous
"""FATM (wavelet spiking module) Trainium2 Bass kernel.

Data-parallel over B across 8 NeuronCores (B=8 -> 1 sample/core).

Per-core pipeline (layout algebra validated in numpy vs the reference):
  chunk-serial over 4 chunks of 128 channels, t-serial over T=4:
    LIF1 (A-layout, fp32 state, scalar_tensor_tensor decay update)
    spikes written twice: zero-padded tile (conv taps) + flat tile;
    all four 32x32-block stream-transposes run on flat unit-stride APs --
    the free-dim permutes they would otherwise need are absorbed into the
    ACT PSUM-evacuation copies (strided output APs are free there).
    fwd Haar: col-transform matmul, evac+permute, ST, row-transform into
    the NegIF1 PSUM accumulator. NegIF spikes: two ACT Sign ops
    (s~ = sign(v-1)+sign(v+1) = 2s), soft reset via -0.5*I matmul feedback.
    bn0+LIF2 in transformed layout via broadcast param tiles (bn0 bias
    folded into thresholds/reset values; fixed-point offset form keeps
    fp16 ranges O(1)). Inverse Haar, then channel-mix (bn1 scale folded in;
    bn1 bias enters as bias1[c]*(Q^T J Q) via a K=1 matmul) -> NegIF2 PSUM
    (A-layout). Conv branch: conv1 folded into conv2 center tap + BN folds;
    9 shifted matmuls over the padded spike tile into the output PSUM,
    plus 0.5*inv2*s~2 (diag matmul) and all biases (K=1 ones matmul).
    Final: out = 2*(x/2) + OUT_psum via one DVE scalar_tensor_tensor.
  Spike reset masks are fp16 spike tiles bitcast to uint16.
"""
import os
import sys
sys.path.insert(0, '/opt/trn_rl_repo')
sys.path.insert(0, '/root/.axon_site/_ro/trn_rl_repo')

import numpy as np

import bass_rust
from concourse import bass, mybir
import concourse.tile as tile_mod
from concourse.tile import TileContext
from concourse.vector_clock import ScopedClock
from concourse.bass_utils import run_bass_kernel_spmd

# ------------------------------------------------------------- walrus fix
MAX_WAITS = 1


def _patched_drain_and_barrier(self, tick_clock, wait_clock):
    drain_inst = self.nc.sync.drain()
    wait_clock.add_sem_waits(
        drain_inst.ins, ScopedClock({None: tick_clock.global_clock})
    )
    si = drain_inst.ins.sync_info
    if si is not None and si.on_wait and len(si.on_wait) > MAX_WAITS:
        waits = list(si.on_wait)
        si.on_wait = waits[:MAX_WAITS]
        for i in range(MAX_WAITS, len(waits), MAX_WAITS):
            nop = self.nc.sync.nop(nofuse=True, hint="wait_spill")
            nop.ins.sync_info = bass_rust.SyncInfo(
                on_wait=waits[i:i + MAX_WAITS], on_update=[]
            )
    self.nc.all_engine_barrier()
    assert self.sems is not None
    popped = self.nc._tile_sem_poison_stack.pop()
    assert popped is self._sem_poison
    self.nc.clear_and_free_semaphores(list(self.sems.allocated().values()))
    self.nc.all_engine_barrier()


tile_mod.TileContext._drain_and_barrier = _patched_drain_and_barrier


def _split_excess_waits(nc):
    """This walrus build rejects >1 sync wait per instruction; spill excess
    waits onto same-engine nops inserted before the instruction."""
    n_split = 0
    for bb in nc.main_func.blocks:
        insts = list(bb.instructions)
        out, changed = [], False
        for ins in insts:
            si = ins.sync_info
            if si is not None and si.on_wait and len(si.on_wait) > MAX_WAITS:
                waits = list(si.on_wait)
                si.on_wait = waits[-MAX_WAITS:]
                for i in range(0, len(waits) - MAX_WAITS, MAX_WAITS):
                    nop = mybir.InstNoOp(name=f"{ins.name}_wsp{i}", ins=[],
                                         outs=[])
                    nop.engine = ins.engine
                    nop.sync_info = bass_rust.SyncInfo(
                        on_wait=waits[i:i + MAX_WAITS], on_update=[])
                    out.append(nop)
                    n_split += 1
                changed = True
            out.append(ins)
        if changed:
            try:
                bb.instructions = out
            except Exception:
                lst = bb.instructions
                lst.clear()
                lst.extend(out)
    return n_split


# ---------------------------------------------------------------- consts
EPS = 1e-5
T, Bb, C, Hh, Ww = 4, 8, 512, 32, 32
NQ, HW = 4, 1024
NCORES = 8
F32 = mybir.dt.float32
F16 = mybir.dt.float16
U16 = mybir.dt.uint16
ALU = mybir.AluOpType
AF = mybir.ActivationFunctionType
TAPS = [(dy, dx) for dy in (-1, 0, 1) for dx in (-1, 0, 1)]

# fp16 [128, x] consts packed into one DRAM array (order defines offsets)
PACK128 = [
    ('lfwdT', 128), ('linvT', 128), ('negIT', 128),
    ('mixT', NQ * 128), ('convT', NQ * 9 * 128), ('a2dT', NQ * 128),
    ('inv0c', NQ * 32), ('thr2', NQ * 32), ('negB', NQ * 32),
]
PACK1 = [('p0row', HW), ('ones', HW), ('beta1', NQ * 128),
         ('betaA', NQ * 128)]


def _haar_matrix(n):
    h = np.array([[1.0]])
    while h.shape[0] < n:
        top = np.kron(h, [1.0, 1.0])
        bot = np.kron(np.eye(h.shape[0]), [1.0, -1.0])
        h = np.concatenate([top, bot], axis=0) / np.sqrt(2.0)
    return h


def _bd4(block_fn):
    L = np.zeros((128, 128))
    for cb in range(4):
        L[32 * cb:32 * cb + 32, 32 * cb:32 * cb + 32] = block_fn(cb)
    return L


def _host_consts(inputs):
    hw_ = np.asarray(inputs['haar_weight'], np.float64)
    w1 = np.asarray(inputs['conv1_w'], np.float64)[:, :, 0, 0]
    b1 = np.asarray(inputs['conv1_b'], np.float64)
    w2 = np.asarray(inputs['conv2_w'], np.float64)
    b2 = np.asarray(inputs['conv2_b'], np.float64)
    bnw = np.asarray(inputs['bn_weight'], np.float64)
    bnb = np.asarray(inputs['bn_bias'], np.float64)
    bnm = np.asarray(inputs['bn_mean'], np.float64)
    bnv = np.asarray(inputs['bn_var'], np.float64)
    inv = bnw / np.sqrt(bnv + EPS)
    bbias = bnb - bnm * inv

    Q = _haar_matrix(32)
    P0flat = (Q.T @ np.ones((32, 32)) @ Q).reshape(HW)

    d = {}
    d['lfwdT'] = _bd4(lambda cb: Q.T)        # [p=(cb,w), m=(cb,l)] = Q[l,w]
    d['linvT'] = _bd4(lambda cb: Q)          # [p=(cb,i), m=(cb,a)] = Q[i,a]
    d['negIT'] = -0.5 * np.eye(128)
    d['p0row'] = P0flat.reshape(1, HW)
    d['ones'] = np.ones((1, HW))

    mixT = np.zeros((NQ, 128, 128))
    convT = np.zeros((NQ, 9, 128, 128))
    a2dT = np.zeros((NQ, 128, 128))
    beta1 = np.zeros((NQ, 128))
    betaA = np.zeros((NQ, 128))
    inv0c = np.zeros((NQ, 128, 32))
    thr2 = np.zeros((NQ, 128, 32))
    negB = np.zeros((NQ, 128, 32))

    for q in range(NQ):
        cidx = np.arange(128 * q, 128 * q + 128)
        inv0q, bias0q = inv[0][cidx], bbias[0][cidx]
        inv1q, bias1q = inv[1][cidx], bbias[1][cidx]
        inv2q, bias2q = inv[2][cidx], bbias[2][cidx]
        inv3q, bias3q = inv[3][cidx], bbias[3][cidx]
        inv4q, bias4q = inv[4][cidx], bbias[4][cidx]

        mixT[q] = _bd4(
            lambda cb: hw_[4 * q + cb] * inv1q.reshape(4, 32)[cb][None, :])
        for ti, (dy, dx) in enumerate(TAPS):
            def cb_blk(cb, dy=dy, dx=dx):
                m = w2[:, :, dy + 1, dx + 1].T * \
                    inv4q.reshape(4, 32)[cb][None, :]
                if dy == 0 and dx == 0:
                    m = m + w1.T * inv3q.reshape(4, 32)[cb][None, :]
                return m
            convT[q, ti] = _bd4(cb_blk)
        a2dT[q] = np.diag(inv2q / 2.0)
        beta1[q] = bias1q
        betaA[q] = (inv4q * np.tile(b2, 16)[cidx] + bias4q
                    + inv3q * np.tile(b1, 16)[cidx] + bias3q + bias2q)

        def cbc(v):
            return np.repeat(v.reshape(4, 32), 32, axis=0)

        inv0c[q] = cbc(inv0q) / 4.0
        thr2[q] = 1.0 - cbc(bias0q)
        negB[q] = -cbc(bias0q)

    d['mixT'] = mixT.transpose(1, 0, 2).reshape(128, NQ * 128)
    d['convT'] = convT.transpose(2, 0, 1, 3).reshape(128, NQ * 9 * 128)
    d['a2dT'] = a2dT.transpose(1, 0, 2).reshape(128, NQ * 128)
    d['inv0c'] = inv0c.transpose(1, 0, 2).reshape(128, NQ * 32)
    d['thr2'] = thr2.transpose(1, 0, 2).reshape(128, NQ * 32)
    d['negB'] = negB.transpose(1, 0, 2).reshape(128, NQ * 32)
    d['beta1'] = beta1.reshape(1, NQ * 128)
    d['betaA'] = betaA.reshape(1, NQ * 128)

    c128 = np.concatenate(
        [np.asarray(d[n]).reshape(128, w) for n, w in PACK128],
        axis=1).astype(np.float16)
    c1 = np.concatenate(
        [np.asarray(d[n]).reshape(1, w) for n, w in PACK1],
        axis=1).astype(np.float16)
    return np.ascontiguousarray(c128), np.ascontiguousarray(c1)


P128_W = sum(w for _, w in PACK128)
P1_W = sum(w for _, w in PACK1)
P128_OFF = {}
_o = 0
for _n, _w in PACK128:
    P128_OFF[_n] = (_o, _w)
    _o += _w
P1_OFF = {}
_o = 0
for _n, _w in PACK1:
    P1_OFF[_n] = (_o, _w)
    _o += _w


def _build_program():
    nc = bass.Bass("TRN2", target_bir_lowering=False, debug=False)
    x32d = nc.declare_dram_parameter("x32h", [NQ, 128, T * HW], F32,
                                     isOutput=False)
    c128d = nc.declare_dram_parameter("c128", [128, P128_W], F16,
                                      isOutput=False)
    c1d = nc.declare_dram_parameter("c1", [1, P1_W], F16, isOutput=False)
    outd = nc.declare_dram_parameter("out", [NQ, 128, T * HW], F32,
                                     isOutput=True)

    with TileContext(nc) as tc:
        with (
            tc.tile_pool(name="consts", bufs=1) as cpool,
            tc.tile_pool(name="xq", bufs=2) as xqpool,
            tc.tile_pool(name="ost", bufs=2) as ostpool,
            tc.tile_pool(name="state", bufs=2) as spool,
            tc.tile_pool(name="spp", bufs=2) as sppool,
            tc.tile_pool(name="work", bufs=3) as wpool,
            tc.tile_pool(name="psV1", bufs=1, space="PSUM") as psV1,
            tc.tile_pool(name="psV2", bufs=1, space="PSUM") as psV2,
            tc.tile_pool(name="pstr", bufs=2, space="PSUM") as pstr,
        ):
            c128 = cpool.tile([128, P128_W], F16, tag="c128")
            nc.sync.dma_start(c128[:, :], c128d.ap())
            c1 = cpool.tile([1, P1_W], F16, tag="c1")
            nc.sync.dma_start(c1[:, :], c1d.ap())
            one_m1 = cpool.tile([128, 1], F32, tag="bm1")
            one_p1 = cpool.tile([128, 1], F32, tag="bp1")
            zero32 = cpool.tile([128, 1], F32, tag="z32")
            nc.vector.memset(one_m1[:, :], -1.0)
            nc.vector.memset(one_p1[:, :], 1.0)
            nc.vector.memset(zero32[:, :], 0.0)

            def c128s(name, idx=0, w=128):
                off, _ = P128_OFF[name]
                return c128[:, off + idx * w: off + (idx + 1) * w]

            def c1s(name, idx=0, w=None):
                off, tot = P1_OFF[name]
                if w is None:
                    w = tot
                return c1[0:1, off + idx * w: off + (idx + 1) * w]

            def bcast(name, q):
                # [128, 32] chunk-param -> [128, 32, 32] stride-0 inner AP
                ap = c128s(name, q, 32)
                return ap.rearrange("p (o a) -> p o a", o=1, a=32).rearrange(
                    "p o a -> p a o").broadcast_to((128, 32, 32))

            v3 = lambda ap: ap.rearrange("p (a b) -> p a b", a=32, b=32)

            for q in range(NQ):
                xq = xqpool.tile([128, T * HW], F32, tag="xq")
                nc.sync.dma_start(xq[:, :], x32d.ap()[q])
                ostage = ostpool.tile([128, T * HW], F32, tag="ost")

                u1 = spool.tile([128, HW], F32, tag="u1")
                u2h = spool.tile([128, HW], F16, tag="u2h")
                nc.gpsimd.memset(u1[:, :], 0.0)
                nc.gpsimd.tensor_copy(v3(u2h[:, :]), bcast('negB', q))
                V1 = psV1.tile([128, HW], F32, tag="V1")
                V2 = psV2.tile([128, HW], F32, tag="V2")

                for t in range(T):
                    xt32 = xq[:, t * HW:(t + 1) * HW]

                    # ---- LIF1: u1 = 0.5*u1 + x/2; spike at 1.0 ----
                    nc.vector.scalar_tensor_tensor(
                        u1[:, :], u1[:, :], 0.5, xt32, ALU.mult, ALU.add)
                    sp = sppool.tile([128, 34 * 34], F16, tag="sp")
                    if q == 0 and t < 2:
                        nc.gpsimd.memset(sp[:, :], 0.0)
                    sp3 = sp[:, :].rearrange("p (h w) -> p h w", h=34, w=34)
                    nc.vector.tensor_scalar(
                        sp3[:, 1:33, 1:33],
                        u1[:, :].rearrange("p (h w) -> p h w", h=32, w=32),
                        1.0, None, ALU.is_ge)
                    sN = wpool.tile([128, HW], F16, tag="sN")
                    nc.vector.tensor_scalar(sN[:, :], u1[:, :], 1.0, None,
                                            ALU.is_ge)
                    nc.vector.copy_predicated(
                        u1[:, :], sN[:, :].bitcast(U16),
                        zero32[:, :].broadcast_to((128, HW)))

                    # ---- ST1 (flat): A -> D [(cb,w), 32h+cc] ----
                    sD = wpool.tile([128, HW], F16, tag="sD")
                    nc.vector.transpose(sD[:, :], sN[:, :])
                    # ---- fwd col MM: M1[(cb,l), 32h+cc] ----
                    M1 = pstr.tile([128, HW], F32, tag="tr")
                    lfwdT = c128s('lfwdT')
                    linvT = c128s('linvT')
                    negIT = c128s('negIT')
                    sD3 = sD[:, :].rearrange("p (h cc) -> p h cc", h=32,
                                             cc=32)
                    for j in (0, 1):
                        nc.tensor.matmul(
                            M1[:, 512 * j:512 * j + 512].rearrange(
                                "p (cc h) -> p h cc", cc=16, h=32),
                            lfwdT, sD3[:, :, 16 * j:16 * j + 16],
                            start=True, stop=True)
                    # ---- plain evac (already permuted to 32cc+h) ----
                    M1s = wpool.tile([128, HW], F16, tag="M1s")
                    nc.scalar.copy(M1s[:, :], M1[:, :])
                    # ---- ST2 (flat): -> B [(cb,h), 32cc+l] ----
                    sB = wpool.tile([128, HW], F16, tag="sB")
                    nc.vector.transpose(sB[:, :], M1s[:, :])
                    # ---- fwd row MM: V1 += [(cb,i), 32cc+l] ----
                    for h_ in (0, 512):
                        nc.tensor.matmul(V1[:, h_:h_ + 512], lfwdT,
                                         sB[:, h_:h_ + 512], start=(t == 0),
                                         stop=(t == T - 1),
                                         skip_group_check=True)

                    # ---- negif1 spikes ----
                    g1 = wpool.tile([128, HW], F16, tag="g1")
                    g2 = wpool.tile([128, HW], F16, tag="g2")
                    nc.scalar.activation(g1[:, :], V1[:, :], AF.Sign,
                                         bias=one_m1[:, :])
                    nc.scalar.activation(g2[:, :], V1[:, :], AF.Sign,
                                         bias=one_p1[:, :])
                    st1 = wpool.tile([128, HW], F16, tag="st1")
                    nc.gpsimd.tensor_tensor(st1[:, :], g1[:, :], g2[:, :],
                                            ALU.add)
                    for h_ in (0, 512):
                        nc.tensor.matmul(V1[:, h_:h_ + 512], negIT,
                                         st1[:, h_:h_ + 512], start=False,
                                         stop=False, skip_group_check=True)

                    # ---- bn0 + LIF2 (transformed layout) ----
                    pprod = wpool.tile([128, HW], F16, tag="pprod")
                    nc.gpsimd.tensor_tensor(v3(pprod[:, :]), v3(st1[:, :]),
                                            bcast('inv0c', q), ALU.mult)
                    nc.vector.scalar_tensor_tensor(
                        u2h[:, :], u2h[:, :], 0.5, pprod[:, :],
                        ALU.mult, ALU.add)
                    s2 = wpool.tile([128, HW], F16, tag="s2")
                    nc.vector.tensor_tensor(v3(s2[:, :]), v3(u2h[:, :]),
                                            bcast('thr2', q), ALU.is_ge)
                    nc.vector.copy_predicated(v3(u2h[:, :]),
                                              v3(s2[:, :].bitcast(U16)),
                                              bcast('negB', q))

                    # ---- inverse Haar: row-inv first ----
                    Z = pstr.tile([128, HW], F32, tag="tr")
                    for h_ in (0, 512):
                        nc.tensor.matmul(Z[:, h_:h_ + 512], linvT,
                                         s2[:, h_:h_ + 512], start=True,
                                         stop=True)
                    Zs = wpool.tile([128, HW], F16, tag="Zs")
                    nc.scalar.copy(Zs[:, :], Z[:, :])
                    ZT = wpool.tile([128, HW], F16, tag="ZT")
                    nc.vector.transpose(ZT[:, :], Zs[:, :])
                    W2 = pstr.tile([128, HW], F32, tag="tr")
                    ZT3 = ZT[:, :].rearrange("p (cc a) -> p cc a", cc=32,
                                             a=32)
                    for j in (0, 1):
                        nc.tensor.matmul(
                            W2[:, 512 * j:512 * j + 512].rearrange(
                                "p (a cc) -> p cc a", a=16, cc=32),
                            linvT, ZT3[:, :, 16 * j:16 * j + 16],
                            start=True, stop=True)
                    Ws = wpool.tile([128, HW], F16, tag="Ws")
                    nc.scalar.copy(Ws[:, :], W2[:, :])
                    haarA = wpool.tile([128, HW], F16, tag="haarA")
                    nc.vector.transpose(haarA[:, :], Ws[:, :])

                    # ---- mix' + P0 bias -> V2 (A-layout) ----
                    for h_ in (0, 512):
                        nc.tensor.matmul(V2[:, h_:h_ + 512],
                                         c128s('mixT', q),
                                         haarA[:, h_:h_ + 512],
                                         start=(t == 0), stop=(t == T - 1),
                                         skip_group_check=True)
                    for h_ in (0, 512):
                        nc.tensor.matmul(V2[:, h_:h_ + 512],
                                         c1s('beta1', q, 128),
                                         c1s('p0row')[0:1, h_:h_ + 512],
                                         start=False, stop=False,
                                         skip_group_check=True)

                    # ---- negif2 ----
                    g1b = wpool.tile([128, HW], F16, tag="g1b")
                    g2b = wpool.tile([128, HW], F16, tag="g2b")
                    nc.scalar.activation(g1b[:, :], V2[:, :], AF.Sign,
                                         bias=one_m1[:, :])
                    nc.scalar.activation(g2b[:, :], V2[:, :], AF.Sign,
                                         bias=one_p1[:, :])
                    st2 = wpool.tile([128, HW], F16, tag="st2")
                    nc.gpsimd.tensor_tensor(st2[:, :], g1b[:, :], g2b[:, :],
                                            ALU.add)
                    for h_ in (0, 512):
                        nc.tensor.matmul(V2[:, h_:h_ + 512], negIT,
                                         st2[:, h_:h_ + 512], start=False,
                                         stop=False, skip_group_check=True)

                    # ---- OUT psum: conv + haar + biases ----
                    OUT = pstr.tile([128, HW], F32, tag="tr")
                    for ti in range(9):
                        dy, dx = TAPS[ti]
                        cT = c128s('convT', q * 9 + ti)
                        rhs = sp3[:, 1 + dy:33 + dy, 1 + dx:33 + dx]
                        nc.tensor.matmul(OUT[:, 0:512], cT, rhs[:, 0:16, :],
                                         start=(ti == 0), stop=False,
                                         skip_group_check=True)
                        nc.tensor.matmul(OUT[:, 512:1024], cT,
                                         rhs[:, 16:32, :],
                                         start=(ti == 0), stop=False,
                                         skip_group_check=True)
                    a2dT = c128s('a2dT', q)
                    for h_ in (0, 512):
                        nc.tensor.matmul(OUT[:, h_:h_ + 512], a2dT,
                                         st2[:, h_:h_ + 512], start=False,
                                         stop=False, skip_group_check=True)
                        nc.tensor.matmul(OUT[:, h_:h_ + 512],
                                         c1s('betaA', q, 128),
                                         c1s('ones')[0:1, h_:h_ + 512],
                                         start=False, stop=True,
                                         skip_group_check=True)

                    # ---- final: out = 2*(x/2) + OUT ----
                    nc.vector.scalar_tensor_tensor(
                        ostage[:, t * HW:(t + 1) * HW], xt32, 2.0,
                        OUT[:, :], ALU.mult, ALU.add)

                nc.sync.dma_start(outd.ap()[q], ostage[:, :])

    _split_excess_waits(nc)
    return nc


_NC_CACHE = None


def _get_nc():
    global _NC_CACHE
    if _NC_CACHE is None:
        _NC_CACHE = _build_program()
    return _NC_CACHE


def kernel(**inputs):
    x = np.asarray(inputs['x'], np.float32)          # [T, B, C, H, W]
    c128, c1 = _host_consts(inputs)

    in_maps = []
    for b in range(NCORES):
        # [T, C, HW] -> [NQ, 128, T*HW], halved for the LIF1 decay form
        xb = (0.5 * x[:, b]).reshape(T, NQ, 128, HW).transpose(1, 2, 0, 3)
        m = {'x32h': np.ascontiguousarray(xb).reshape(NQ, 128, T * HW)
             .astype(np.float32),
             'c128': c128, 'c1': c1}
        in_maps.append(m)

    nc = _get_nc()
    res = run_bass_kernel_spmd(nc, in_maps, list(range(NCORES))).results
    # out [NQ, 128, T*HW] -> [T, B, C, H, W]
    outs = []
    for b in range(NCORES):
        ob = res[b]['out'].reshape(NQ, 128, T, HW).transpose(2, 0, 1, 3)
        outs.append(ob.reshape(T, C, HW))
    out = np.stack(outs, axis=1)
    return out.reshape(T, Bb, C, Hh, Ww).astype(np.float32)


# revision 12
# speedup vs baseline: 1.5997x; 1.0111x over previous
"""FATM (wavelet spiking module) Trainium2 Bass kernel.

Data-parallel over B across 8 NeuronCores (B=8 -> 1 sample/core).

Per-core pipeline (layout algebra validated in numpy vs the reference):
  chunk-serial over 4 chunks of 128 channels, t-serial over T=4:
    LIF1 (A-layout, fp32 state, scalar_tensor_tensor decay update)
    spikes written twice: zero-padded tile (conv taps) + flat tile;
    all four 32x32-block stream-transposes run on flat unit-stride APs --
    the free-dim permutes they would otherwise need are absorbed into the
    ACT PSUM-evacuation copies (strided output APs are free there).
    fwd Haar: col-transform matmul, evac+permute, ST, row-transform into
    the NegIF1 PSUM accumulator. NegIF spikes: two ACT Sign ops
    (s~ = sign(v-1)+sign(v+1) = 2s), soft reset via -0.5*I matmul feedback.
    bn0+LIF2 in transformed layout via broadcast param tiles (bn0 bias
    folded into thresholds/reset values; fixed-point offset form keeps
    fp16 ranges O(1)). Inverse Haar, then channel-mix (bn1 scale folded in;
    bn1 bias enters as bias1[c]*(Q^T J Q) via a K=1 matmul) -> NegIF2 PSUM
    (A-layout). Conv branch: conv1 folded into conv2 center tap + BN folds;
    9 shifted matmuls over the padded spike tile into the output PSUM,
    plus 0.5*inv2*s~2 (diag matmul) and all biases (K=1 ones matmul).
    Final: out = 2*(x/2) + OUT_psum via one DVE scalar_tensor_tensor.
  Spike reset masks are fp16 spike tiles bitcast to uint16.
"""
import os
import sys
sys.path.insert(0, '/opt/trn_rl_repo')
sys.path.insert(0, '/root/.axon_site/_ro/trn_rl_repo')

import numpy as np

import bass_rust
from concourse import bass, mybir
import concourse.tile as tile_mod
from concourse.tile import TileContext
from concourse.vector_clock import ScopedClock
from concourse.bass_utils import run_bass_kernel_spmd

# ------------------------------------------------------------- walrus fix
MAX_WAITS = 1


def _patched_drain_and_barrier(self, tick_clock, wait_clock):
    drain_inst = self.nc.sync.drain()
    wait_clock.add_sem_waits(
        drain_inst.ins, ScopedClock({None: tick_clock.global_clock})
    )
    si = drain_inst.ins.sync_info
    if si is not None and si.on_wait and len(si.on_wait) > MAX_WAITS:
        waits = list(si.on_wait)
        si.on_wait = waits[:MAX_WAITS]
        for i in range(MAX_WAITS, len(waits), MAX_WAITS):
            nop = self.nc.sync.nop(nofuse=True, hint="wait_spill")
            nop.ins.sync_info = bass_rust.SyncInfo(
                on_wait=waits[i:i + MAX_WAITS], on_update=[]
            )
    self.nc.all_engine_barrier()
    assert self.sems is not None
    popped = self.nc._tile_sem_poison_stack.pop()
    assert popped is self._sem_poison
    self.nc.clear_and_free_semaphores(list(self.sems.allocated().values()))
    self.nc.all_engine_barrier()


tile_mod.TileContext._drain_and_barrier = _patched_drain_and_barrier


def _split_excess_waits(nc):
    """This walrus build rejects >1 sync wait per instruction; spill excess
    waits onto same-engine nops inserted before the instruction."""
    n_split = 0
    for bb in nc.main_func.blocks:
        insts = list(bb.instructions)
        out, changed = [], False
        for ins in insts:
            si = ins.sync_info
            if si is not None and si.on_wait and len(si.on_wait) > MAX_WAITS:
                waits = list(si.on_wait)
                si.on_wait = waits[-MAX_WAITS:]
                for i in range(0, len(waits) - MAX_WAITS, MAX_WAITS):
                    nop = mybir.InstNoOp(name=f"{ins.name}_wsp{i}", ins=[],
                                         outs=[])
                    nop.engine = ins.engine
                    nop.sync_info = bass_rust.SyncInfo(
                        on_wait=waits[i:i + MAX_WAITS], on_update=[])
                    out.append(nop)
                    n_split += 1
                changed = True
            out.append(ins)
        if changed:
            try:
                bb.instructions = out
            except Exception:
                lst = bb.instructions
                lst.clear()
                lst.extend(out)
    return n_split


# ---------------------------------------------------------------- consts
EPS = 1e-5
T, Bb, C, Hh, Ww = 4, 8, 512, 32, 32
NQ, HW = 4, 1024
NCORES = 8
F32 = mybir.dt.float32
F16 = mybir.dt.float16
U16 = mybir.dt.uint16
ALU = mybir.AluOpType
AF = mybir.ActivationFunctionType
TAPS = [(dy, dx) for dy in (-1, 0, 1) for dx in (-1, 0, 1)]

# fp16 [128, x] consts packed into one DRAM array (order defines offsets)
PACK128 = [
    ('lfwdT', 128), ('linvT', 128), ('negIT', 128),
    ('mixT', NQ * 128), ('convT', NQ * 9 * 128), ('a2dT', NQ * 128),
    ('inv0c', NQ * 32), ('thr2', NQ * 32), ('negB', NQ * 32),
]
PACK1 = [('p0row', HW), ('ones', HW), ('beta1', NQ * 128),
         ('betaA', NQ * 128)]


def _haar_matrix(n):
    h = np.array([[1.0]])
    while h.shape[0] < n:
        top = np.kron(h, [1.0, 1.0])
        bot = np.kron(np.eye(h.shape[0]), [1.0, -1.0])
        h = np.concatenate([top, bot], axis=0) / np.sqrt(2.0)
    return h


def _bd4(block_fn):
    L = np.zeros((128, 128))
    for cb in range(4):
        L[32 * cb:32 * cb + 32, 32 * cb:32 * cb + 32] = block_fn(cb)
    return L


def _host_consts(inputs):
    hw_ = np.asarray(inputs['haar_weight'], np.float64)
    w1 = np.asarray(inputs['conv1_w'], np.float64)[:, :, 0, 0]
    b1 = np.asarray(inputs['conv1_b'], np.float64)
    w2 = np.asarray(inputs['conv2_w'], np.float64)
    b2 = np.asarray(inputs['conv2_b'], np.float64)
    bnw = np.asarray(inputs['bn_weight'], np.float64)
    bnb = np.asarray(inputs['bn_bias'], np.float64)
    bnm = np.asarray(inputs['bn_mean'], np.float64)
    bnv = np.asarray(inputs['bn_var'], np.float64)
    inv = bnw / np.sqrt(bnv + EPS)
    bbias = bnb - bnm * inv

    Q = _haar_matrix(32)
    P0flat = (Q.T @ np.ones((32, 32)) @ Q).reshape(HW)

    d = {}
    d['lfwdT'] = _bd4(lambda cb: Q.T)        # [p=(cb,w), m=(cb,l)] = Q[l,w]
    d['linvT'] = _bd4(lambda cb: Q)          # [p=(cb,i), m=(cb,a)] = Q[i,a]
    d['negIT'] = -0.5 * np.eye(128)
    d['p0row'] = P0flat.reshape(1, HW)
    d['ones'] = np.ones((1, HW))

    mixT = np.zeros((NQ, 128, 128))
    convT = np.zeros((NQ, 9, 128, 128))
    a2dT = np.zeros((NQ, 128, 128))
    beta1 = np.zeros((NQ, 128))
    betaA = np.zeros((NQ, 128))
    inv0c = np.zeros((NQ, 128, 32))
    thr2 = np.zeros((NQ, 128, 32))
    negB = np.zeros((NQ, 128, 32))

    for q in range(NQ):
        cidx = np.arange(128 * q, 128 * q + 128)
        inv0q, bias0q = inv[0][cidx], bbias[0][cidx]
        inv1q, bias1q = inv[1][cidx], bbias[1][cidx]
        inv2q, bias2q = inv[2][cidx], bbias[2][cidx]
        inv3q, bias3q = inv[3][cidx], bbias[3][cidx]
        inv4q, bias4q = inv[4][cidx], bbias[4][cidx]

        mixT[q] = _bd4(
            lambda cb: hw_[4 * q + cb] * inv1q.reshape(4, 32)[cb][None, :])
        for ti, (dy, dx) in enumerate(TAPS):
            def cb_blk(cb, dy=dy, dx=dx):
                m = w2[:, :, dy + 1, dx + 1].T * \
                    inv4q.reshape(4, 32)[cb][None, :]
                if dy == 0 and dx == 0:
                    m = m + w1.T * inv3q.reshape(4, 32)[cb][None, :]
                return m
            convT[q, ti] = _bd4(cb_blk)
        a2dT[q] = np.diag(inv2q / 2.0)
        beta1[q] = bias1q
        betaA[q] = (inv4q * np.tile(b2, 16)[cidx] + bias4q
                    + inv3q * np.tile(b1, 16)[cidx] + bias3q + bias2q)

        def cbc(v):
            return np.repeat(v.reshape(4, 32), 32, axis=0)

        inv0c[q] = cbc(inv0q) / 4.0
        thr2[q] = 1.0 - cbc(bias0q)
        negB[q] = -cbc(bias0q)

    d['mixT'] = mixT.transpose(1, 0, 2).reshape(128, NQ * 128)
    d['convT'] = convT.transpose(2, 0, 1, 3).reshape(128, NQ * 9 * 128)
    d['a2dT'] = a2dT.transpose(1, 0, 2).reshape(128, NQ * 128)
    d['inv0c'] = inv0c.transpose(1, 0, 2).reshape(128, NQ * 32)
    d['thr2'] = thr2.transpose(1, 0, 2).reshape(128, NQ * 32)
    d['negB'] = negB.transpose(1, 0, 2).reshape(128, NQ * 32)
    d['beta1'] = beta1.reshape(1, NQ * 128)
    d['betaA'] = betaA.reshape(1, NQ * 128)

    c128 = np.concatenate(
        [np.asarray(d[n]).reshape(128, w) for n, w in PACK128],
        axis=1).astype(np.float16)
    c1 = np.concatenate(
        [np.asarray(d[n]).reshape(1, w) for n, w in PACK1],
        axis=1).astype(np.float16)
    return np.ascontiguousarray(c128), np.ascontiguousarray(c1)


P128_W = sum(w for _, w in PACK128)
P1_W = sum(w for _, w in PACK1)
P128_OFF = {}
_o = 0
for _n, _w in PACK128:
    P128_OFF[_n] = (_o, _w)
    _o += _w
P1_OFF = {}
_o = 0
for _n, _w in PACK1:
    P1_OFF[_n] = (_o, _w)
    _o += _w


def _build_program():
    nc = bass.Bass("TRN2", target_bir_lowering=False, debug=False)
    x32d = nc.declare_dram_parameter("x32h", [NQ, 128, T * HW], F32,
                                     isOutput=False)
    c128d = nc.declare_dram_parameter("c128", [128, P128_W], F16,
                                      isOutput=False)
    c1d = nc.declare_dram_parameter("c1", [1, P1_W], F16, isOutput=False)
    outd = nc.declare_dram_parameter("out", [NQ, 128, T * HW], F32,
                                     isOutput=True)

    with TileContext(nc) as tc:
        with (
            tc.tile_pool(name="consts", bufs=1) as cpool,
            tc.tile_pool(name="xq", bufs=2) as xqpool,
            tc.tile_pool(name="ost", bufs=2) as ostpool,
            tc.tile_pool(name="state", bufs=2) as spool,
            tc.tile_pool(name="spp", bufs=2) as sppool,
            tc.tile_pool(name="work", bufs=3) as wpool,
            tc.tile_pool(name="psV1", bufs=1, space="PSUM") as psV1,
            tc.tile_pool(name="psV2", bufs=1, space="PSUM") as psV2,
            tc.tile_pool(name="pstr", bufs=2, space="PSUM") as pstr,
        ):
            c128 = cpool.tile([128, P128_W], F16, tag="c128")
            nc.sync.dma_start(c128[:, :], c128d.ap())
            c1 = cpool.tile([1, P1_W], F16, tag="c1")
            nc.sync.dma_start(c1[:, :], c1d.ap())
            one_m1 = cpool.tile([128, 1], F32, tag="bm1")
            one_p1 = cpool.tile([128, 1], F32, tag="bp1")
            zero32 = cpool.tile([128, 1], F32, tag="z32")
            nc.vector.memset(one_m1[:, :], -1.0)
            nc.vector.memset(one_p1[:, :], 1.0)
            nc.vector.memset(zero32[:, :], 0.0)

            def c128s(name, idx=0, w=128):
                off, _ = P128_OFF[name]
                return c128[:, off + idx * w: off + (idx + 1) * w]

            def c1s(name, idx=0, w=None):
                off, tot = P1_OFF[name]
                if w is None:
                    w = tot
                return c1[0:1, off + idx * w: off + (idx + 1) * w]

            def bcast(name, q):
                # [128, 32] chunk-param -> [128, 32, 32] stride-0 inner AP
                ap = c128s(name, q, 32)
                return ap.rearrange("p (o a) -> p o a", o=1, a=32).rearrange(
                    "p o a -> p a o").broadcast_to((128, 32, 32))

            v3 = lambda ap: ap.rearrange("p (a b) -> p a b", a=32, b=32)

            for q in range(NQ):
                xq = xqpool.tile([128, T * HW], F32, tag="xq")
                nc.sync.dma_start(xq[:, :], x32d.ap()[q])

                u1 = spool.tile([128, HW], F32, tag="u1")
                u2h = spool.tile([128, HW], F16, tag="u2h")
                nc.gpsimd.memset(u1[:, :], 0.0)
                nc.gpsimd.tensor_copy(v3(u2h[:, :]), bcast('negB', q))
                V1 = psV1.tile([128, HW], F32, tag="V1")
                V2 = psV2.tile([128, HW], F32, tag="V2")

                for t in range(T):
                    xt32 = xq[:, t * HW:(t + 1) * HW]

                    # ---- LIF1: u1 = 0.5*u1 + x/2; spike at 1.0 ----
                    nc.vector.scalar_tensor_tensor(
                        u1[:, :], u1[:, :], 0.5, xt32, ALU.mult, ALU.add)
                    sp = sppool.tile([128, 34 * 34], F16, tag="sp")
                    if q == 0 and t < 2:
                        nc.gpsimd.memset(sp[:, :], 0.0)
                    sp3 = sp[:, :].rearrange("p (h w) -> p h w", h=34, w=34)
                    nc.vector.tensor_scalar(
                        sp3[:, 1:33, 1:33],
                        u1[:, :].rearrange("p (h w) -> p h w", h=32, w=32),
                        1.0, None, ALU.is_ge)
                    sN = wpool.tile([128, HW], F16, tag="sN")
                    nc.vector.tensor_scalar(sN[:, :], u1[:, :], 1.0, None,
                                            ALU.is_ge)
                    nc.vector.copy_predicated(
                        u1[:, :], sN[:, :].bitcast(U16),
                        zero32[:, :].broadcast_to((128, HW)))

                    # ---- ST1 (flat): A -> D [(cb,w), 32h+cc] ----
                    sD = wpool.tile([128, HW], F16, tag="sD")
                    nc.vector.transpose(sD[:, :], sN[:, :])
                    # ---- fwd col MM: M1[(cb,l), 32h+cc] ----
                    M1 = pstr.tile([128, HW], F32, tag="tr")
                    lfwdT = c128s('lfwdT')
                    linvT = c128s('linvT')
                    negIT = c128s('negIT')
                    sD3 = sD[:, :].rearrange("p (h cc) -> p h cc", h=32,
                                             cc=32)
                    for j in (0, 1):
                        nc.tensor.matmul(
                            M1[:, 512 * j:512 * j + 512].rearrange(
                                "p (cc h) -> p h cc", cc=16, h=32),
                            lfwdT, sD3[:, :, 16 * j:16 * j + 16],
                            start=True, stop=True)
                    # ---- plain evac (already permuted to 32cc+h) ----
                    M1s = wpool.tile([128, HW], F16, tag="M1s")
                    nc.scalar.copy(M1s[:, :], M1[:, :])
                    # ---- ST2 (flat): -> B [(cb,h), 32cc+l] ----
                    sB = wpool.tile([128, HW], F16, tag="sB")
                    nc.vector.transpose(sB[:, :], M1s[:, :])
                    # ---- fwd row MM: V1 += [(cb,i), 32cc+l] ----
                    for h_ in (0, 512):
                        nc.tensor.matmul(V1[:, h_:h_ + 512], lfwdT,
                                         sB[:, h_:h_ + 512], start=(t == 0),
                                         stop=(t == T - 1),
                                         skip_group_check=True)

                    # ---- negif1 spikes ----
                    g1 = wpool.tile([128, HW], F16, tag="g1")
                    g2 = wpool.tile([128, HW], F16, tag="g2")
                    nc.scalar.activation(g1[:, :], V1[:, :], AF.Sign,
                                         bias=one_m1[:, :])
                    nc.scalar.activation(g2[:, :], V1[:, :], AF.Sign,
                                         bias=one_p1[:, :])
                    st1 = wpool.tile([128, HW], F16, tag="st1")
                    nc.gpsimd.tensor_tensor(st1[:, :], g1[:, :], g2[:, :],
                                            ALU.add)
                    for h_ in (0, 512):
                        nc.tensor.matmul(V1[:, h_:h_ + 512], negIT,
                                         st1[:, h_:h_ + 512], start=False,
                                         stop=False, skip_group_check=True)

                    # ---- bn0 + LIF2 (transformed layout) ----
                    pprod = wpool.tile([128, HW], F16, tag="pprod")
                    nc.gpsimd.tensor_tensor(v3(pprod[:, :]), v3(st1[:, :]),
                                            bcast('inv0c', q), ALU.mult)
                    nc.vector.scalar_tensor_tensor(
                        u2h[:, :], u2h[:, :], 0.5, pprod[:, :],
                        ALU.mult, ALU.add)
                    s2 = wpool.tile([128, HW], F16, tag="s2")
                    nc.vector.tensor_tensor(v3(s2[:, :]), v3(u2h[:, :]),
                                            bcast('thr2', q), ALU.is_ge)
                    nc.vector.copy_predicated(v3(u2h[:, :]),
                                              v3(s2[:, :].bitcast(U16)),
                                              bcast('negB', q))

                    # ---- inverse Haar: row-inv first ----
                    Z = pstr.tile([128, HW], F32, tag="tr")
                    for h_ in (0, 512):
                        nc.tensor.matmul(Z[:, h_:h_ + 512], linvT,
                                         s2[:, h_:h_ + 512], start=True,
                                         stop=True)
                    Zs = wpool.tile([128, HW], F16, tag="Zs")
                    nc.scalar.copy(Zs[:, :], Z[:, :])
                    ZT = wpool.tile([128, HW], F16, tag="ZT")
                    nc.vector.transpose(ZT[:, :], Zs[:, :])
                    W2 = pstr.tile([128, HW], F32, tag="tr")
                    ZT3 = ZT[:, :].rearrange("p (cc a) -> p cc a", cc=32,
                                             a=32)
                    for j in (0, 1):
                        nc.tensor.matmul(
                            W2[:, 512 * j:512 * j + 512].rearrange(
                                "p (a cc) -> p cc a", a=16, cc=32),
                            linvT, ZT3[:, :, 16 * j:16 * j + 16],
                            start=True, stop=True)
                    Ws = wpool.tile([128, HW], F16, tag="Ws")
                    nc.scalar.copy(Ws[:, :], W2[:, :])
                    haarA = wpool.tile([128, HW], F16, tag="haarA")
                    nc.vector.transpose(haarA[:, :], Ws[:, :])

                    # ---- mix' + P0 bias -> V2 (A-layout) ----
                    for h_ in (0, 512):
                        nc.tensor.matmul(V2[:, h_:h_ + 512],
                                         c128s('mixT', q),
                                         haarA[:, h_:h_ + 512],
                                         start=(t == 0), stop=(t == T - 1),
                                         skip_group_check=True)
                    for h_ in (0, 512):
                        nc.tensor.matmul(V2[:, h_:h_ + 512],
                                         c1s('beta1', q, 128),
                                         c1s('p0row')[0:1, h_:h_ + 512],
                                         start=False, stop=False,
                                         skip_group_check=True)

                    # ---- negif2 ----
                    g1b = wpool.tile([128, HW], F16, tag="g1b")
                    g2b = wpool.tile([128, HW], F16, tag="g2b")
                    nc.scalar.activation(g1b[:, :], V2[:, :], AF.Sign,
                                         bias=one_m1[:, :])
                    nc.scalar.activation(g2b[:, :], V2[:, :], AF.Sign,
                                         bias=one_p1[:, :])
                    st2 = wpool.tile([128, HW], F16, tag="st2")
                    nc.gpsimd.tensor_tensor(st2[:, :], g1b[:, :], g2b[:, :],
                                            ALU.add)
                    for h_ in (0, 512):
                        nc.tensor.matmul(V2[:, h_:h_ + 512], negIT,
                                         st2[:, h_:h_ + 512], start=False,
                                         stop=False, skip_group_check=True)

                    # ---- OUT psum: conv + haar + biases ----
                    OUT = pstr.tile([128, HW], F32, tag="tr")
                    for ti in range(9):
                        dy, dx = TAPS[ti]
                        cT = c128s('convT', q * 9 + ti)
                        rhs = sp3[:, 1 + dy:33 + dy, 1 + dx:33 + dx]
                        nc.tensor.matmul(OUT[:, 0:512], cT, rhs[:, 0:16, :],
                                         start=(ti == 0), stop=False,
                                         skip_group_check=True)
                        nc.tensor.matmul(OUT[:, 512:1024], cT,
                                         rhs[:, 16:32, :],
                                         start=(ti == 0), stop=False,
                                         skip_group_check=True)
                    a2dT = c128s('a2dT', q)
                    for h_ in (0, 512):
                        nc.tensor.matmul(OUT[:, h_:h_ + 512], a2dT,
                                         st2[:, h_:h_ + 512], start=False,
                                         stop=False, skip_group_check=True)
                        nc.tensor.matmul(OUT[:, h_:h_ + 512],
                                         c1s('betaA', q, 128),
                                         c1s('ones')[0:1, h_:h_ + 512],
                                         start=False, stop=True,
                                         skip_group_check=True)

                    # ---- final: out = 2*(x/2) + OUT ----
                    osb = ostpool.tile([128, HW], F32, tag="ost")
                    nc.vector.scalar_tensor_tensor(
                        osb[:, :], xt32, 2.0, OUT[:, :], ALU.mult, ALU.add)
                    nc.sync.dma_start(
                        outd.ap()[q][:, t * HW:(t + 1) * HW], osb[:, :])

    _split_excess_waits(nc)
    return nc


_NC_CACHE = None


def _get_nc():
    global _NC_CACHE
    if _NC_CACHE is None:
        _NC_CACHE = _build_program()
    return _NC_CACHE


def kernel(**inputs):
    x = np.asarray(inputs['x'], np.float32)          # [T, B, C, H, W]
    c128, c1 = _host_consts(inputs)

    in_maps = []
    for b in range(NCORES):
        # [T, C, HW] -> [NQ, 128, T*HW], halved for the LIF1 decay form
        xb = (0.5 * x[:, b]).reshape(T, NQ, 128, HW).transpose(1, 2, 0, 3)
        m = {'x32h': np.ascontiguousarray(xb).reshape(NQ, 128, T * HW)
             .astype(np.float32),
             'c128': c128, 'c1': c1}
        in_maps.append(m)

    nc = _get_nc()
    res = run_bass_kernel_spmd(nc, in_maps, list(range(NCORES))).results
    # out [NQ, 128, T*HW] -> [T, B, C, H, W]
    outs = []
    for b in range(NCORES):
        ob = res[b]['out'].reshape(NQ, 128, T, HW).transpose(2, 0, 1, 3)
        outs.append(ob.reshape(T, C, HW))
    out = np.stack(outs, axis=1)
    return out.reshape(T, Bb, C, Hh, Ww).astype(np.float32)
